# revision 14
# baseline (speedup 1.0000x reference)
"""EvoAttn (V-only causal self-attention) on 8 Trainium2 NeuronCores.

Full input x:(2,2048,2048) fp32 -> full output (2,2048,2048) fp32.
Sharding: 32 (b,h) head-slices, 4 per core (head parallel).

Per (b,h) on-device (L=2048, D=128), with V = x[b,:,h*128:(h+1)*128]:
  S^T tiles  : psum[k=128, q<=1024] = VT[:,kblk].T @ VT[:,qchunk]  (bf16 PE)
  causal mask: diagonal 128x128 sub-block multiplied by host mask (GpSimd)
  E^T tiles  : rows with q_local <  SB -> exp(S^T/sqrt(D)) on ScalarE
               rows with q_local >= SB -> Schraudolph exp2 bit-trick on DVE:
                 bf16_bits = int16(round(S * C1 + C2)) ~= exp(S/sqrt(D))
               (split is per-q-row so each output row uses one consistent
               approximation; the softmax ratio cancels the systematic err)
  PV         : psum strips [q=128, 129] += E^T[:,qsub].T @ Vaug[kblk]; four
               q-blocks share one 2-bank psum group tile (ones col -> denom)
  normalize  : one batched DVE reciprocal + one broadcast tensor_tensor
               multiply per 4-block group, out to SBUF fp32, DMA per
               half-head on the GpSimd queue
"""

import sys

for _p in ("/opt/trn_rl_repo",):
    if _p not in sys.path:
        sys.path.insert(0, _p)

import numpy as np
import ml_dtypes

BF16 = ml_dtypes.bfloat16

B, L, E = 2, 2048, 2048
H, D = 16, 128
P = 128          # partition dim / k-block
QC = 1024        # q chunk (two PSUM banks of fp32)
NKB = L // P     # 16 k-blocks
NQC = L // QC    # 2 q chunks
QB = QC // P     # 8 q-blocks per chunk
NCORES = 8
H4 = (B * H) // NCORES  # 4 heads per core
SCALE = 1.0 / float(np.sqrt(D))

# Schraudolph exp2 constants: bf16 bits = round(S*C1 + C2) gives
# ~exp(S/sqrt(D)) with ~3% max rel err; rows >= S_BOUND use this path.
C1 = float(128.0 * np.log2(np.e) * SCALE)
C2 = float(127 * 128 - 42)
S_BOUND = 832    # q_local < S_BOUND -> ScalarE exp; >= -> DVE bit-trick

_cache = {}


def _build_nc():
    import concourse.bacc as bacc
    import concourse.mybir as mybir
    import concourse.tile as tile
    from contextlib import ExitStack

    f32 = mybir.dt.float32
    bf16 = mybir.dt.bfloat16
    i16 = mybir.dt.int16

    nc = bacc.Bacc("TRN2", target_bir_lowering=False, debug=False,
                   num_devices=NCORES)

    x_vq = nc.dram_tensor("vq", [H4, 2, P, QB * (D + 1)], bf16,
                          kind="ExternalInput")
    x_vt = nc.dram_tensor("vt", [H4, 4, D, 512], bf16, kind="ExternalInput")
    x_mask = nc.dram_tensor("mask", [P, P], bf16, kind="ExternalInput")
    y = nc.dram_tensor("y", [H4, L, D], f32, kind="ExternalOutput")

    EXP = mybir.ActivationFunctionType.Exp
    MULT = mybir.AluOpType.mult
    ADD = mybir.AluOpType.add

    with tile.TileContext(nc) as tc, ExitStack() as ctx:
        const_pool = ctx.enter_context(tc.tile_pool(name="const", bufs=1))
        vq_pool = ctx.enter_context(tc.tile_pool(name="vq", bufs=2))
        vt_pool = ctx.enter_context(tc.tile_pool(name="vt", bufs=2))
        pt_pool = ctx.enter_context(tc.tile_pool(name="pt", bufs=48))
        out_pool = ctx.enter_context(tc.tile_pool(name="out", bufs=2))
        rec_pool = ctx.enter_context(tc.tile_pool(name="rec", bufs=4))
        ps_s = ctx.enter_context(tc.tile_pool(name="ps_s", bufs=3, space="PSUM"))
        ps_o = ctx.enter_context(tc.tile_pool(name="ps_o", bufs=2, space="PSUM"))

        mask_t = const_pool.tile([P, P], bf16)
        nc.gpsimd.dma_start(mask_t[:], x_mask[:, :])

        # preamble: load the exp ACT table + warm the PE HAM clock gate
        # while the first head's input DMAs are in flight (warm weights come
        # from a locally memset tile so nothing waits on a DMA)
        warm_a = rec_pool.tile([P, 1], f32, tag="warm_a")
        warm_b = rec_pool.tile([P, 1], f32, tag="warm_b")
        warm_w = const_pool.tile([P, P], bf16)
        nc.vector.memset(warm_a[:], 0.0)
        nc.vector.memset(warm_w[:], 0.0)
        nc.scalar.activation(warm_b[:], warm_a[:], EXP, scale=1.0)
        warm_ps = ps_s.tile([P, QC], f32, tag="ps_s")
        for _ in range(16):
            nc.tensor.matmul(warm_ps[:, 0:P], warm_w[:], warm_w[:],
                             start=True, stop=True)

        vt_ts, vq_ts, o_bigs = {}, {}, {}
        pt_tiles = {}

        def load_head(h):
            vt_t = vt_pool.tile([P, L], bf16, tag="vt")
            vq_t = vq_pool.tile([P, NKB, D + 1], bf16, tag="vq")
            for c in range(4):  # vt first: the first scores tile needs it
                nc.sync.dma_start(vt_t[:, c * 512:(c + 1) * 512],
                                  x_vt[h][c])
            for c in range(2):
                nc.gpsimd.dma_start(
                    vq_t[:, c * QB:(c + 1) * QB, :],
                    x_vq[h][c].rearrange("p (kb c) -> p kb c", kb=QB),
                )
            vt_ts[h], vq_ts[h] = vt_t, vq_t
            o_bigs[h] = out_pool.tile([P, NKB, D], f32, tag="obig", name="obig")

        def emit_scores_tile(h, qc, kb):
            j = kb - QB * qc  # >=0 -> diagonal-chunk block
            off = max(0, j) * P
            vt_t = vt_ts[h]
            lhs = vt_t[:, kb * P:(kb + 1) * P]
            q0 = qc * QC
            ps = ps_s.tile([P, QC], f32, tag="ps_s")
            if off < 512:
                nc.tensor.matmul(ps[:, off:512], lhs,
                                 vt_t[:, q0 + off:q0 + 512],
                                 start=True, stop=True)
                nc.tensor.matmul(ps[:, 512:], lhs,
                                 vt_t[:, q0 + 512:q0 + QC],
                                 start=True, stop=True)
            else:
                nc.tensor.matmul(ps[:, off:], lhs,
                                 vt_t[:, q0 + off:q0 + QC],
                                 start=True, stop=True)
            pt = pt_pool.tile([P, QC], bf16, tag="pt")
            mid = max(off, S_BOUND)
            if mid > off:
                nc.scalar.activation(pt[:, off:mid], ps[:, off:mid], EXP,
                                     scale=SCALE)
            if QC > mid:
                nc.vector.tensor_scalar(pt[:, mid:QC].bitcast(i16),
                                        ps[:, mid:QC], C1, C2, MULT, ADD)
            if j >= 0:
                nc.vector.tensor_mul(pt[:, off:off + P],
                                     pt[:, off:off + P], mask_t[:])
            pt_tiles[(h, qc, kb)] = pt

        # PV chain work drained as individual matmuls from a FIFO so each
        # scores tile is followed by just enough PV matmuls to fill PE's
        # slack while ACT/DVE run exp. A backlog floor keeps work in
        # reserve for the chain-less pass-1-early windows.
        chain_fifo = []   # (h, qc, qi) in completion order
        cur = {"mm": 0, "po": None}
        backlog = {"mms": 0}
        po_groups = {}

        def strip_ap(po, l):
            return po[:, 129 * l:129 * l + 129]

        def finish_group(h, g, po):
            # batched reciprocal: denom cols at [[129,2]] offset 128
            rec = rec_pool.tile([P, 2], f32, tag="rec")
            den = po[:].rearrange("p (s y) -> p s y", s=2)[:, :, 128]
            nc.vector.reciprocal(rec[:].rearrange("p (s y) -> p s y", y=1),
                                 den)
            src = po[:].rearrange("p (s y) -> p s y", s=2)[:, :, 0:128]
            recb = rec[:].unsqueeze(2).to_broadcast((P, 2, 128))
            dst = o_bigs[h][:, 2 * g:2 * g + 2, :]
            nc.vector.tensor_tensor(dst, src, recb, MULT)
            if g % 2 == 1:  # quarter-head ready -> drain it
                qq = g // 2
                nc.gpsimd.dma_start(
                    y[h][qq * 512:(qq + 1) * 512, :].rearrange(
                        "(kb p) d -> p kb d", p=P),
                    o_bigs[h][:, qq * 4:(qq + 1) * 4, :],
                )

        def emit_chain_mms(n):
            while n > 0 and chain_fifo:
                h, qc, qi = chain_fifo[0]
                qsub = qi - QB * qc
                g, l = qi // 2, qi % 2
                if cur["po"] is None:
                    if l == 0:
                        po_groups[(h, g)] = ps_o.tile([P, 258], f32,
                                                      tag="ps_o", name="po")
                    cur["po"] = po_groups[(h, g)]
                    cur["mm"] = 0
                kb = cur["mm"]
                nc.tensor.matmul(
                    strip_ap(cur["po"], l),
                    pt_tiles[(h, qc, kb)][:, qsub * P:(qsub + 1) * P],
                    vq_ts[h][:, kb, :],
                    start=(kb == 0), stop=(kb == qi),
                )
                cur["mm"] += 1
                backlog["mms"] -= 1
                n -= 1
                if cur["mm"] == qi + 1:
                    if l == 1:
                        finish_group(h, g, cur["po"])
                    chain_fifo.pop(0)
                    cur["po"] = None

        # chain matmuls run faster in uninterrupted streaks; accumulate the
        # per-tile budget and emit in bursts of ~2 tiles' worth
        FLOOR = 56
        BURST = 8
        budget_acc = {"n": 0}
        for h in range(H4):
            load_head(h)
            for qc in range(NQC):
                for kb in range(QB * qc + QB):
                    j = kb - QB * qc
                    in_p1_early = qc == NQC - 1 and j < 0
                    floor = 0 if (h == H4 - 1 or in_p1_early) else FLOOR
                    emit_scores_tile(h, qc, kb)
                    if j >= 0:
                        qi = QB * qc + j
                        chain_fifo.append((h, qc, qi))
                        backlog["mms"] += qi + 1
                    cols = QC - max(0, j) * P
                    budget_acc["n"] += cols // 190 + 2
                    if budget_acc["n"] >= BURST:
                        emit_chain_mms(
                            min(budget_acc["n"], backlog["mms"] - floor))
                        budget_acc["n"] = 0
        emit_chain_mms(backlog["mms"])

    nc.compile()
    return nc


def _get_nc():
    if "nc" not in _cache:
        _cache["nc"] = _build_nc()
    return _cache["nc"]


def _make_mask():
    # keep (partition=k_local, free=q_local) where q_local >= k_local
    pk = np.arange(P)[:, None]
    fq = np.arange(P)[None, :]
    return (fq >= pk).astype(BF16)


def kernel(x):
    from concourse.bass_utils import run_bass_kernel_spmd

    x = np.asarray(x)
    in_dtype = x.dtype
    assert x.shape == (B, L, E)

    nc = _get_nc()

    # (B, L, H, D) -> (B*H, L, D), bf16
    v = np.ascontiguousarray(
        x.reshape(B, L, H, D).transpose(0, 2, 1, 3)
    ).reshape(B * H, L, D).astype(BF16)

    mask = _make_mask()
    in_maps = []
    for c in range(NCORES):
        sl = v[H4 * c:H4 * (c + 1)]                      # (H4, L, D)
        # chunk-major vq: [H4, 2, P, QB*(D+1)], ones column appended
        vq = np.ones((H4, P, NKB, D + 1), dtype=BF16)
        vq[..., :D] = sl.reshape(H4, NKB, P, D).transpose(0, 2, 1, 3)
        vq = np.ascontiguousarray(
            vq.reshape(H4, P, 2, QB * (D + 1)).transpose(0, 2, 1, 3))
        # chunk-major vt: [H4, 4, D, 512]
        vt = sl.transpose(0, 2, 1).reshape(H4, D, 4, 512)
        vt = np.ascontiguousarray(vt.transpose(0, 2, 1, 3))
        in_maps.append({"vq": vq, "vt": vt, "mask": mask})

    import os

    kwargs = {}
    if os.environ.get("KERNEL_TRACE"):
        kwargs["trace"] = True
        if os.environ.get("KERNEL_TRACE_DIR"):
            kwargs["tmpdir"] = os.environ["KERNEL_TRACE_DIR"]
    res = run_bass_kernel_spmd(nc, in_maps, core_ids=list(range(NCORES)), **kwargs)
    _cache["last_results"] = res
    ys = np.stack([res.results[c]["y"] for c in range(NCORES)], axis=0)
    # (NCORES, H4, L, D) -> (B, H, L, D) -> (B, L, E)
    out = ys.reshape(B, H, L, D).transpose(0, 2, 1, 3).reshape(B, L, E)
    return out.astype(in_dtype, copy=False)


# revision 15
# speedup vs baseline: 1.0058x; 1.0058x over previous
"""EvoAttn (V-only causal self-attention) on 8 Trainium2 NeuronCores.

Full input x:(2,2048,2048) fp32 -> full output (2,2048,2048) fp32.
Sharding: 32 (b,h) head-slices, 4 per core (head parallel).

Per (b,h) on-device (L=2048, D=128), with V = x[b,:,h*128:(h+1)*128]:
  S^T tiles  : psum[k=128, q<=1024] = VT[:,kblk].T @ VT[:,qchunk]  (bf16 PE)
  causal mask: diagonal 128x128 sub-block multiplied by host mask (GpSimd)
  E^T tiles  : rows with q_local <  SB -> exp(S^T/sqrt(D)) on ScalarE
               rows with q_local >= SB -> Schraudolph exp2 bit-trick on DVE:
                 bf16_bits = int16(round(S * C1 + C2)) ~= exp(S/sqrt(D))
               (split is per-q-row so each output row uses one consistent
               approximation; the softmax ratio cancels the systematic err)
  PV         : psum strips [q=128, 129] += E^T[:,qsub].T @ Vaug[kblk]; four
               q-blocks share one 2-bank psum group tile (ones col -> denom)
  normalize  : one batched DVE reciprocal + one broadcast tensor_tensor
               multiply per 4-block group, out to SBUF fp32, DMA per
               half-head on the GpSimd queue
"""

import sys

for _p in ("/opt/trn_rl_repo",):
    if _p not in sys.path:
        sys.path.insert(0, _p)

import numpy as np
import ml_dtypes

BF16 = ml_dtypes.bfloat16

B, L, E = 2, 2048, 2048
H, D = 16, 128
P = 128          # partition dim / k-block
QC = 1024        # q chunk (two PSUM banks of fp32)
NKB = L // P     # 16 k-blocks
NQC = L // QC    # 2 q chunks
QB = QC // P     # 8 q-blocks per chunk
NCORES = 8
H4 = (B * H) // NCORES  # 4 heads per core
SCALE = 1.0 / float(np.sqrt(D))

# Schraudolph exp2 constants: bf16 bits = round(S*C1 + C2) gives
# ~exp(S/sqrt(D)) with ~3% max rel err; rows >= S_BOUND use this path.
C1 = float(128.0 * np.log2(np.e) * SCALE)
C2 = float(127 * 128 - 42)
S_BOUND = 832    # q_local < S_BOUND -> ScalarE exp; >= -> DVE bit-trick

_cache = {}


def _build_nc():
    import concourse.bacc as bacc
    import concourse.mybir as mybir
    import concourse.tile as tile
    from contextlib import ExitStack

    f32 = mybir.dt.float32
    bf16 = mybir.dt.bfloat16
    i16 = mybir.dt.int16

    nc = bacc.Bacc("TRN2", target_bir_lowering=False, debug=False,
                   num_devices=NCORES)

    x_vq = nc.dram_tensor("vq", [H4, 2, P, QB * (D + 1)], bf16,
                          kind="ExternalInput")
    x_vt = nc.dram_tensor("vt", [H4, 4, D, 512], bf16, kind="ExternalInput")
    x_mask = nc.dram_tensor("mask", [P, P], bf16, kind="ExternalInput")
    y = nc.dram_tensor("y", [H4, L, D], f32, kind="ExternalOutput")

    EXP = mybir.ActivationFunctionType.Exp
    MULT = mybir.AluOpType.mult
    ADD = mybir.AluOpType.add

    with tile.TileContext(nc) as tc, ExitStack() as ctx:
        const_pool = ctx.enter_context(tc.tile_pool(name="const", bufs=1))
        vq_pool = ctx.enter_context(tc.tile_pool(name="vq", bufs=2))
        vt_pool = ctx.enter_context(tc.tile_pool(name="vt", bufs=2))
        pt_pool = ctx.enter_context(tc.tile_pool(name="pt", bufs=48))
        out_pool = ctx.enter_context(tc.tile_pool(name="out", bufs=2))
        rec_pool = ctx.enter_context(tc.tile_pool(name="rec", bufs=4))
        ps_s = ctx.enter_context(tc.tile_pool(name="ps_s", bufs=3, space="PSUM"))
        ps_o = ctx.enter_context(tc.tile_pool(name="ps_o", bufs=2, space="PSUM"))

        mask_t = const_pool.tile([P, P], bf16)
        nc.gpsimd.dma_start(mask_t[:], x_mask[:, :])

        # preamble: load the exp ACT table + warm the PE HAM clock gate
        # while the first head's input DMAs are in flight (warm weights come
        # from a locally memset tile so nothing waits on a DMA)
        warm_a = rec_pool.tile([P, 1], f32, tag="warm_a")
        warm_b = rec_pool.tile([P, 1], f32, tag="warm_b")
        warm_w = const_pool.tile([P, P], bf16)
        nc.vector.memset(warm_a[:], 0.0)
        nc.vector.memset(warm_w[:], 0.0)
        nc.scalar.activation(warm_b[:], warm_a[:], EXP, scale=1.0)
        warm_ps = ps_s.tile([P, QC], f32, tag="ps_s")
        for _ in range(16):
            nc.tensor.matmul(warm_ps[:, 0:P], warm_w[:], warm_w[:],
                             start=True, stop=True)

        vt_ts, vq_ts, o_bigs = {}, {}, {}
        pt_tiles = {}

        def load_head(h):
            vt_t = vt_pool.tile([P, L], bf16, tag="vt")
            vq_t = vq_pool.tile([P, NKB, D + 1], bf16, tag="vq")
            for c in range(4):  # vt first: the first scores tile needs it
                nc.sync.dma_start(vt_t[:, c * 512:(c + 1) * 512],
                                  x_vt[h][c])
            for c in range(2):
                nc.gpsimd.dma_start(
                    vq_t[:, c * QB:(c + 1) * QB, :],
                    x_vq[h][c].rearrange("p (kb c) -> p kb c", kb=QB),
                )
            vt_ts[h], vq_ts[h] = vt_t, vq_t
            o_bigs[h] = out_pool.tile([P, NKB, D], f32, tag="obig", name="obig")

        def emit_scores_tile(h, qc, kb):
            j = kb - QB * qc  # >=0 -> diagonal-chunk block
            off = max(0, j) * P
            vt_t = vt_ts[h]
            lhs = vt_t[:, kb * P:(kb + 1) * P]
            q0 = qc * QC
            ps = ps_s.tile([P, QC], f32, tag="ps_s")
            if off < 512:
                nc.tensor.matmul(ps[:, off:512], lhs,
                                 vt_t[:, q0 + off:q0 + 512],
                                 start=True, stop=True)
                nc.tensor.matmul(ps[:, 512:], lhs,
                                 vt_t[:, q0 + 512:q0 + QC],
                                 start=True, stop=True)
            else:
                nc.tensor.matmul(ps[:, off:], lhs,
                                 vt_t[:, q0 + off:q0 + QC],
                                 start=True, stop=True)
            pt = pt_pool.tile([P, QC], bf16, tag="pt")
            mid = max(off, S_BOUND)
            if mid > off:
                nc.scalar.activation(pt[:, off:mid], ps[:, off:mid], EXP,
                                     scale=SCALE)
            if QC > mid:
                nc.vector.tensor_scalar(pt[:, mid:QC].bitcast(i16),
                                        ps[:, mid:QC], C1, C2, MULT, ADD)
            if j >= 0:
                nc.vector.tensor_mul(pt[:, off:off + P],
                                     pt[:, off:off + P], mask_t[:])
            pt_tiles[(h, qc, kb)] = pt

        # PV chain work drained as individual matmuls from a FIFO so each
        # scores tile is followed by just enough PV matmuls to fill PE's
        # slack while ACT/DVE run exp. A backlog floor keeps work in
        # reserve for the chain-less pass-1-early windows.
        chain_fifo = []   # (h, qc, qi) in completion order
        cur = {"mm": 0, "po": None}
        backlog = {"mms": 0}
        po_groups = {}

        def strip_ap(po, l):
            return po[:, 129 * l:129 * l + 129]

        def finish_group(h, g, po):
            # batched reciprocal: denom cols at [[129,2]] offset 128
            rec = rec_pool.tile([P, 2], f32, tag="rec")
            den = po[:].rearrange("p (s y) -> p s y", s=2)[:, :, 128]
            nc.vector.reciprocal(rec[:].rearrange("p (s y) -> p s y", y=1),
                                 den)
            src = po[:].rearrange("p (s y) -> p s y", s=2)[:, :, 0:128]
            recb = rec[:].unsqueeze(2).to_broadcast((P, 2, 128))
            dst = o_bigs[h][:, 2 * g:2 * g + 2, :]
            nc.vector.tensor_tensor(dst, src, recb, MULT)
            if g % 2 == 1:  # quarter-head ready -> drain it
                qq = g // 2
                nc.gpsimd.dma_start(
                    y[h][qq * 512:(qq + 1) * 512, :].rearrange(
                        "(kb p) d -> p kb d", p=P),
                    o_bigs[h][:, qq * 4:(qq + 1) * 4, :],
                )

        def emit_chain_mms(n):
            while n > 0 and chain_fifo:
                h, qc, qi = chain_fifo[0]
                qsub = qi - QB * qc
                g, l = qi // 2, qi % 2
                if cur["po"] is None:
                    if l == 0:
                        po_groups[(h, g)] = ps_o.tile([P, 258], f32,
                                                      tag="ps_o", name="po")
                    cur["po"] = po_groups[(h, g)]
                    cur["mm"] = 0
                kb = cur["mm"]
                nc.tensor.matmul(
                    strip_ap(cur["po"], l),
                    pt_tiles[(h, qc, kb)][:, qsub * P:(qsub + 1) * P],
                    vq_ts[h][:, kb, :],
                    start=(kb == 0), stop=(kb == qi),
                )
                cur["mm"] += 1
                backlog["mms"] -= 1
                n -= 1
                if cur["mm"] == qi + 1:
                    if l == 1:
                        finish_group(h, g, cur["po"])
                    chain_fifo.pop(0)
                    cur["po"] = None

        # chain matmuls run faster in uninterrupted streaks; accumulate the
        # per-tile budget and emit in bursts of ~2 tiles' worth
        FLOOR = 56
        BURST = 16
        budget_acc = {"n": 0}
        for h in range(H4):
            load_head(h)
            for qc in range(NQC):
                for kb in range(QB * qc + QB):
                    j = kb - QB * qc
                    in_p1_early = qc == NQC - 1 and j < 0
                    floor = 0 if (h == H4 - 1 or in_p1_early) else FLOOR
                    emit_scores_tile(h, qc, kb)
                    if j >= 0:
                        qi = QB * qc + j
                        chain_fifo.append((h, qc, qi))
                        backlog["mms"] += qi + 1
                    cols = QC - max(0, j) * P
                    budget_acc["n"] += cols // 190 + 2
                    if budget_acc["n"] >= BURST:
                        emit_chain_mms(
                            min(budget_acc["n"], backlog["mms"] - floor))
                        budget_acc["n"] = 0
        emit_chain_mms(backlog["mms"])

    nc.compile()
    return nc


def _get_nc():
    if "nc" not in _cache:
        _cache["nc"] = _build_nc()
    return _cache["nc"]


def _make_mask():
    # keep (partition=k_local, free=q_local) where q_local >= k_local
    pk = np.arange(P)[:, None]
    fq = np.arange(P)[None, :]
    return (fq >= pk).astype(BF16)


def kernel(x):
    from concourse.bass_utils import run_bass_kernel_spmd

    x = np.asarray(x)
    in_dtype = x.dtype
    assert x.shape == (B, L, E)

    nc = _get_nc()

    # (B, L, H, D) -> (B*H, L, D), bf16
    v = np.ascontiguousarray(
        x.reshape(B, L, H, D).transpose(0, 2, 1, 3)
    ).reshape(B * H, L, D).astype(BF16)

    mask = _make_mask()
    in_maps = []
    for c in range(NCORES):
        sl = v[H4 * c:H4 * (c + 1)]                      # (H4, L, D)
        # chunk-major vq: [H4, 2, P, QB*(D+1)], ones column appended
        vq = np.ones((H4, P, NKB, D + 1), dtype=BF16)
        vq[..., :D] = sl.reshape(H4, NKB, P, D).transpose(0, 2, 1, 3)
        vq = np.ascontiguousarray(
            vq.reshape(H4, P, 2, QB * (D + 1)).transpose(0, 2, 1, 3))
        # chunk-major vt: [H4, 4, D, 512]
        vt = sl.transpose(0, 2, 1).reshape(H4, D, 4, 512)
        vt = np.ascontiguousarray(vt.transpose(0, 2, 1, 3))
        in_maps.append({"vq": vq, "vt": vt, "mask": mask})

    import os

    kwargs = {}
    if os.environ.get("KERNEL_TRACE"):
        kwargs["trace"] = True
        if os.environ.get("KERNEL_TRACE_DIR"):
            kwargs["tmpdir"] = os.environ["KERNEL_TRACE_DIR"]
    res = run_bass_kernel_spmd(nc, in_maps, core_ids=list(range(NCORES)), **kwargs)
    _cache["last_results"] = res
    ys = np.stack([res.results[c]["y"] for c in range(NCORES)], axis=0)
    # (NCORES, H4, L, D) -> (B, H, L, D) -> (B, L, E)
    out = ys.reshape(B, H, L, D).transpose(0, 2, 1, 3).reshape(B, L, E)
    return out.astype(in_dtype, copy=False)


# revision 17
# speedup vs baseline: 1.0125x; 1.0067x over previous
"""EvoAttn (V-only causal self-attention) on 8 Trainium2 NeuronCores.

Full input x:(2,2048,2048) fp32 -> full output (2,2048,2048) fp32.
Sharding: 32 (b,h) head-slices, 4 per core (head parallel).

Per (b,h) on-device (L=2048, D=128), with V = x[b,:,h*128:(h+1)*128]:
  S^T tiles  : psum[k=128, q<=1024] = VT[:,kblk].T @ VT[:,qchunk]  (bf16 PE)
  causal mask: diagonal 128x128 sub-block multiplied by host mask (GpSimd)
  E^T tiles  : rows with q_local <  SB -> exp(S^T/sqrt(D)) on ScalarE
               rows with q_local >= SB -> Schraudolph exp2 bit-trick on DVE:
                 bf16_bits = int16(round(S * C1 + C2)) ~= exp(S/sqrt(D))
               (split is per-q-row so each output row uses one consistent
               approximation; the softmax ratio cancels the systematic err)
  PV         : psum strips [q=128, 129] += E^T[:,qsub].T @ Vaug[kblk]; four
               q-blocks share one 2-bank psum group tile (ones col -> denom)
  normalize  : one batched DVE reciprocal + one broadcast tensor_tensor
               multiply per 4-block group, out to SBUF fp32, DMA per
               half-head on the GpSimd queue
"""

import sys

for _p in ("/opt/trn_rl_repo",):
    if _p not in sys.path:
        sys.path.insert(0, _p)

import numpy as np
import ml_dtypes

BF16 = ml_dtypes.bfloat16

B, L, E = 2, 2048, 2048
H, D = 16, 128
P = 128          # partition dim / k-block
QC = 1024        # q chunk (two PSUM banks of fp32)
NKB = L // P     # 16 k-blocks
NQC = L // QC    # 2 q chunks
QB = QC // P     # 8 q-blocks per chunk
NCORES = 8
H4 = (B * H) // NCORES  # 4 heads per core
SCALE = 1.0 / float(np.sqrt(D))

# Schraudolph exp2 constants: bf16 bits = round(S*C1 + C2) gives
# ~exp(S/sqrt(D)) with ~3% max rel err; rows >= S_BOUND use this path.
C1 = float(128.0 * np.log2(np.e) * SCALE)
C2 = float(127 * 128 - 42)
S_BOUND = 832    # q_local < S_BOUND -> ScalarE exp; >= -> DVE bit-trick

_cache = {}


def _build_nc():
    import concourse.bacc as bacc
    import concourse.mybir as mybir
    import concourse.tile as tile
    from contextlib import ExitStack

    f32 = mybir.dt.float32
    bf16 = mybir.dt.bfloat16
    i16 = mybir.dt.int16

    nc = bacc.Bacc("TRN2", target_bir_lowering=False, debug=False,
                   num_devices=NCORES)

    x_vq = nc.dram_tensor("vq", [H4, 2, P, QB * (D + 1)], bf16,
                          kind="ExternalInput")
    x_vt = nc.dram_tensor("vt", [H4, 4, D, 512], bf16, kind="ExternalInput")
    x_mask = nc.dram_tensor("mask", [P, P], bf16, kind="ExternalInput")
    y = nc.dram_tensor("y", [H4, L, D], f32, kind="ExternalOutput")

    EXP = mybir.ActivationFunctionType.Exp
    MULT = mybir.AluOpType.mult
    ADD = mybir.AluOpType.add

    with tile.TileContext(nc) as tc, ExitStack() as ctx:
        const_pool = ctx.enter_context(tc.tile_pool(name="const", bufs=1))
        vq_pool = ctx.enter_context(tc.tile_pool(name="vq", bufs=2))
        vt_pool = ctx.enter_context(tc.tile_pool(name="vt", bufs=2))
        pt_pool = ctx.enter_context(tc.tile_pool(name="pt", bufs=48))
        out_pool = ctx.enter_context(tc.tile_pool(name="out", bufs=2))
        rec_pool = ctx.enter_context(tc.tile_pool(name="rec", bufs=4))
        ps_s = ctx.enter_context(tc.tile_pool(name="ps_s", bufs=3, space="PSUM"))
        ps_o = ctx.enter_context(tc.tile_pool(name="ps_o", bufs=2, space="PSUM"))

        mask_t = const_pool.tile([P, P], bf16)
        nc.gpsimd.dma_start(mask_t[:], x_mask[:, :])

        # preamble: load the exp ACT table + warm the PE HAM clock gate
        # while the first head's input DMAs are in flight (warm weights come
        # from a locally memset tile so nothing waits on a DMA)
        warm_a = rec_pool.tile([P, 1], f32, tag="warm_a")
        warm_b = rec_pool.tile([P, 1], f32, tag="warm_b")
        warm_w = const_pool.tile([P, P], bf16)
        nc.vector.memset(warm_a[:], 0.0)
        nc.vector.memset(warm_w[:], 0.0)
        nc.scalar.activation(warm_b[:], warm_a[:], EXP, scale=1.0)
        warm_ps = ps_s.tile([P, QC], f32, tag="ps_s")
        for _ in range(16):
            nc.tensor.matmul(warm_ps[:, 0:P], warm_w[:], warm_w[:],
                             start=True, stop=True)

        vt_ts, vq_ts, o_bigs = {}, {}, {}
        pt_tiles = {}

        def load_head(h):
            vt_t = vt_pool.tile([P, L], bf16, tag="vt")
            vq_t = vq_pool.tile([P, NKB, D + 1], bf16, tag="vq")
            for c in range(4):  # vt first: the first scores tile needs it
                nc.sync.dma_start(vt_t[:, c * 512:(c + 1) * 512],
                                  x_vt[h][c])
            for c in range(2):
                nc.gpsimd.dma_start(
                    vq_t[:, c * QB:(c + 1) * QB, :],
                    x_vq[h][c].rearrange("p (kb c) -> p kb c", kb=QB),
                )
            vt_ts[h], vq_ts[h] = vt_t, vq_t
            o_bigs[h] = out_pool.tile([P, NKB, D], f32, tag="obig", name="obig")

        def emit_scores_tile(h, qc, kb):
            j = kb - QB * qc  # >=0 -> diagonal-chunk block
            off = max(0, j) * P
            vt_t = vt_ts[h]
            lhs = vt_t[:, kb * P:(kb + 1) * P]
            q0 = qc * QC
            ps = ps_s.tile([P, QC], f32, tag="ps_s")
            if off < 512:
                nc.tensor.matmul(ps[:, off:512], lhs,
                                 vt_t[:, q0 + off:q0 + 512],
                                 start=True, stop=True)
                nc.tensor.matmul(ps[:, 512:], lhs,
                                 vt_t[:, q0 + 512:q0 + QC],
                                 start=True, stop=True)
            else:
                nc.tensor.matmul(ps[:, off:], lhs,
                                 vt_t[:, q0 + off:q0 + QC],
                                 start=True, stop=True)
            pt = pt_pool.tile([P, QC], bf16, tag="pt")
            mid = max(off, S_BOUND)
            if mid > off:
                nc.scalar.activation(pt[:, off:mid], ps[:, off:mid], EXP,
                                     scale=SCALE)
            if QC > mid:
                nc.vector.tensor_scalar(pt[:, mid:QC].bitcast(i16),
                                        ps[:, mid:QC], C1, C2, MULT, ADD)
            if j >= 0:
                nc.vector.tensor_mul(pt[:, off:off + P],
                                     pt[:, off:off + P], mask_t[:])
            pt_tiles[(h, qc, kb)] = pt

        # PV chain work drained as individual matmuls from a FIFO so each
        # scores tile is followed by just enough PV matmuls to fill PE's
        # slack while ACT/DVE run exp. A backlog floor keeps work in
        # reserve for the chain-less pass-1-early windows.
        chain_fifo = []   # (h, qc, qi) in completion order
        cur = {"mm": 0, "po": None}
        backlog = {"mms": 0}
        po_groups = {}

        def strip_ap(po, l):
            return po[:, 129 * l:129 * l + 129]

        pending_groups = []

        def flush_groups():
            while pending_groups:
                finish_group(*pending_groups.pop(0))

        def finish_group(h, g, po):
            # batched reciprocal: denom cols at [[129,2]] offset 128
            rec = rec_pool.tile([P, 2], f32, tag="rec")
            den = po[:].rearrange("p (s y) -> p s y", s=2)[:, :, 128]
            nc.vector.reciprocal(rec[:].rearrange("p (s y) -> p s y", y=1),
                                 den)
            src = po[:].rearrange("p (s y) -> p s y", s=2)[:, :, 0:128]
            recb = rec[:].unsqueeze(2).to_broadcast((P, 2, 128))
            dst = o_bigs[h][:, 2 * g:2 * g + 2, :]
            nc.vector.tensor_tensor(dst, src, recb, MULT)
            if g % 2 == 1:  # quarter-head ready -> drain it
                qq = g // 2
                nc.gpsimd.dma_start(
                    y[h][qq * 512:(qq + 1) * 512, :].rearrange(
                        "(kb p) d -> p kb d", p=P),
                    o_bigs[h][:, qq * 4:(qq + 1) * 4, :],
                )

        def emit_chain_mms(n):
            while n > 0 and chain_fifo:
                h, qc, qi = chain_fifo[0]
                qsub = qi - QB * qc
                g, l = qi // 2, qi % 2
                if cur["po"] is None:
                    if l == 0:
                        po_groups[(h, g)] = ps_o.tile([P, 258], f32,
                                                      tag="ps_o", name="po")
                    cur["po"] = po_groups[(h, g)]
                    cur["mm"] = 0
                kb = cur["mm"]
                nc.tensor.matmul(
                    strip_ap(cur["po"], l),
                    pt_tiles[(h, qc, kb)][:, qsub * P:(qsub + 1) * P],
                    vq_ts[h][:, kb, :],
                    start=(kb == 0), stop=(kb == qi),
                )
                cur["mm"] += 1
                backlog["mms"] -= 1
                n -= 1
                if cur["mm"] == qi + 1:
                    if l == 1:
                        pending_groups.append((h, g, cur["po"]))
                    chain_fifo.pop(0)
                    cur["po"] = None

        # chain matmuls run faster in uninterrupted streaks; accumulate the
        # per-tile budget and emit in bursts of ~2 tiles' worth
        FLOOR = 56
        BURST = 12
        budget_acc = {"n": 0}
        for h in range(H4):
            load_head(h)
            for qc in range(NQC):
                for kb in range(QB * qc + QB):
                    j = kb - QB * qc
                    in_p1_early = qc == NQC - 1 and j < 0
                    floor = 0 if (h == H4 - 1 or in_p1_early) else FLOOR
                    emit_scores_tile(h, qc, kb)
                    flush_groups()
                    if j >= 0:
                        qi = QB * qc + j
                        chain_fifo.append((h, qc, qi))
                        backlog["mms"] += qi + 1
                    cols = QC - max(0, j) * P
                    budget_acc["n"] += cols // 190 + 2
                    if budget_acc["n"] >= BURST:
                        emit_chain_mms(
                            min(budget_acc["n"], backlog["mms"] - floor))
                        budget_acc["n"] = 0
        emit_chain_mms(backlog["mms"])
        flush_groups()

    nc.compile()
    return nc


def _get_nc():
    if "nc" not in _cache:
        _cache["nc"] = _build_nc()
    return _cache["nc"]


def _make_mask():
    # keep (partition=k_local, free=q_local) where q_local >= k_local
    pk = np.arange(P)[:, None]
    fq = np.arange(P)[None, :]
    return (fq >= pk).astype(BF16)


def kernel(x):
    from concourse.bass_utils import run_bass_kernel_spmd

    x = np.asarray(x)
    in_dtype = x.dtype
    assert x.shape == (B, L, E)

    nc = _get_nc()

    # (B, L, H, D) -> (B*H, L, D), bf16
    v = np.ascontiguousarray(
        x.reshape(B, L, H, D).transpose(0, 2, 1, 3)
    ).reshape(B * H, L, D).astype(BF16)

    mask = _make_mask()
    in_maps = []
    for c in range(NCORES):
        sl = v[H4 * c:H4 * (c + 1)]                      # (H4, L, D)
        # chunk-major vq: [H4, 2, P, QB*(D+1)], ones column appended
        vq = np.ones((H4, P, NKB, D + 1), dtype=BF16)
        vq[..., :D] = sl.reshape(H4, NKB, P, D).transpose(0, 2, 1, 3)
        vq = np.ascontiguousarray(
            vq.reshape(H4, P, 2, QB * (D + 1)).transpose(0, 2, 1, 3))
        # chunk-major vt: [H4, 4, D, 512]
        vt = sl.transpose(0, 2, 1).reshape(H4, D, 4, 512)
        vt = np.ascontiguousarray(vt.transpose(0, 2, 1, 3))
        in_maps.append({"vq": vq, "vt": vt, "mask": mask})

    import os

    kwargs = {}
    if os.environ.get("KERNEL_TRACE"):
        kwargs["trace"] = True
        if os.environ.get("KERNEL_TRACE_DIR"):
            kwargs["tmpdir"] = os.environ["KERNEL_TRACE_DIR"]
    res = run_bass_kernel_spmd(nc, in_maps, core_ids=list(range(NCORES)), **kwargs)
    _cache["last_results"] = res
    ys = np.stack([res.results[c]["y"] for c in range(NCORES)], axis=0)
    # (NCORES, H4, L, D) -> (B, H, L, D) -> (B, L, E)
    out = ys.reshape(B, H, L, D).transpose(0, 2, 1, 3).reshape(B, L, E)
    return out.astype(in_dtype, copy=False)
